# revision 9
# baseline (speedup 1.0000x reference)
"""Trainium2 Bass kernel for nn_Attention (B=4, P=2048, D=768, H=12, hd=64).

Sharding: 8 cores = 4 batches x 2 half-head-groups (6 heads each).

Pipeline (per core): the Scalar engine's exp stream (~0.53 cyc/elem +
~900 cyc/instruction -> ~186us at 1536-col units) and the PE matmul stream
(~190us) are nearly equal, so the kernel runs one attention pipeline that
keeps both saturated end to end:
  - chunk = (head pair p, q-quarter qq); 12 chunks; 11 exp units per chunk
  - unit u: 3 score MMs (512-col blocks; the pair's heads alternate array
    row halves 0-63/64-127 and overlap on the PE) -> [128, 1536] PSUM ->
    one Exp ACTIVATE into the bf16 slab
  - o^T for chunk c-1 drains during chunk c (~3 MMs/unit, heads
    sequential so one aux-psum slot suffices), denominator via a
    ones-column in the V stationary (softmax without max subtraction,
    |s| <= 9.2); VG=128 stationary keeps fast-weight-load enabled
  - qkv q/k features, v projection and output projection stream through a
    deadline-tagged fill queue consumed in leftover PE slots each unit
PSUM: scores 2x[128,1536] (6 banks) + one 2-slot [128,512] aux pool shared
by the o^T accumulators and the fill-job tiles.
Host sums the two half-head partials per batch and transposes back.
"""

import sys

import numpy as np

if "/opt/trn_rl_repo" not in sys.path:
    sys.path.insert(0, "/opt/trn_rl_repo")

B, P, D = 4, 2048, 768
H, HD = 12, 64
N_CORES = 8
H_LOC = 6
SCALE = HD ** -0.5

CC = 6         # contraction chunks of 128 over D=768
KT = 16        # k-position tiles of 128 over P=2048
PT = 16        # token tiles of 128
TB = 4         # token blocks of 512
VG = 128
VW = H_LOC * (HD + 1)  # 390

UNIT = 1536
NBLK = 2 * KT          # 32 512-blocks per chunk
TOTAL = NBLK * 512
N_UNITS = (TOTAL + UNIT - 1) // UNIT  # 11 (last = 1024)

_PROG = None


def _build_program():
    import concourse.mybir as mybir
    import concourse.tile as tile
    from concourse import bacc

    f32 = mybir.dt.float32
    bf16 = mybir.dt.bfloat16
    AF = mybir.ActivationFunctionType

    nc = bacc.Bacc("TRN2")

    xT = nc.declare_dram_parameter("xT", [769, 2048], bf16, isOutput=False)
    wqk = nc.declare_dram_parameter("wqk", [768, 768], bf16, isOutput=False)
    wv = nc.declare_dram_parameter("wv", [769, VW], bf16, isOutput=False)
    wp = nc.declare_dram_parameter("wp", [384, 768], bf16, isOutput=False)
    bqk = nc.declare_dram_parameter("bqk", [128, 6], f32, isOutput=False)
    bp = nc.declare_dram_parameter("bp", [128, 6], f32, isOutput=False)
    yT = nc.declare_dram_parameter("yT", [768, 2048], f32, isOutput=True)

    with tile.TileContext(nc) as tc:
        with (
            tc.tile_pool(name="persist", bufs=1) as persist,
            tc.tile_pool(name="slabs", bufs=2) as slabs,
            tc.tile_pool(name="norm", bufs=3) as norm,
            tc.tile_pool(name="drs", bufs=4, space="DRAM") as drs,
            tc.tile_pool(name="psum_s", bufs=2, space="PSUM") as psum_s,
            tc.tile_pool(name="aux", bufs=2, space="PSUM") as aux,
        ):
            # ---- persistent SBUF tensors ----
            qkt = persist.tile([128, 6, 2048], bf16, tag="qkt")
            vsb = persist.tile([128, KT, H_LOC * VG], bf16, tag="vsb")
            nc.vector.memset(
                vsb.rearrange("p a (h g) -> p a h g", g=VG)[:, :, :, 1:64], 0.0)
            otsb = persist.tile([128, 3, 2048], bf16, tag="otsb")
            xt = persist.tile([128, CC, 2048], bf16, tag="xt")
            xt1 = persist.tile([1, 2048], bf16, tag="xt1")
            wqk_sb = persist.tile([128, CC, 6, 128], bf16, tag="wqk_sb")
            wv_sb = persist.tile([128, CC, VW], bf16, tag="wv_sb")
            wv1 = persist.tile([1, VW], bf16, tag="wv1")
            wp_sb = persist.tile([128, 3, 768], bf16, tag="wp_sb")
            bqk_sb = persist.tile([128, 6], f32, tag="bqk_sb")
            bp_sb = persist.tile([128, 6], f32, tag="bp_sb")

            nc.sync.dma_start(out=bqk_sb, in_=bqk[:, :])
            nc.sync.dma_start(out=bp_sb, in_=bp[:, :])
            # pre-warm the exp ACT table set (~2.7us) during the DMA lead
            warmup = persist.tile([1, 1], f32, tag="warmup")
            nc.vector.memset(warmup, 0.0)
            nc.scalar.activation(out=warmup, in_=warmup, func=AF.Exp)

            # DMA chase order: per cc bring x, then the k/q features of
            # pair 0 (ft3, ft0) so the first chunk can start ASAP.
            for cc in range(CC):
                nc.sync.dma_start(out=xt[:, cc, :],
                                  in_=xT[cc * 128:(cc + 1) * 128, :])
                for ft in (3, 0):
                    nc.sync.dma_start(
                        out=wqk_sb[:, cc, ft, :],
                        in_=wqk[cc * 128:(cc + 1) * 128,
                                ft * 128:(ft + 1) * 128])
            nc.sync.dma_start(out=xt1, in_=xT[768:769, :])
            for cc in range(CC):
                nc.sync.dma_start(out=wv_sb[:, cc, :],
                                  in_=wv[cc * 128:(cc + 1) * 128, :])
            nc.sync.dma_start(out=wv1, in_=wv[768:769, :])
            for ft in (4, 1, 5, 2):
                for cc in range(CC):
                    nc.sync.dma_start(
                        out=wqk_sb[:, cc, ft, :],
                        in_=wqk[cc * 128:(cc + 1) * 128,
                                ft * 128:(ft + 1) * 128])
            for fc in range(3):
                nc.sync.dma_start(out=wp_sb[:, fc, :],
                                  in_=wp[fc * 128:(fc + 1) * 128, :])

            # ---------------- fill-job machinery ----------------
            # Small closures (<= 2 matmuls each) consumed in per-unit slack.
            fillq = []       # list of (tag, fn)
            done_tags = set()

            def pop_fill(n):
                for _ in range(n):
                    if not fillq:
                        return
                    tag, fn = fillq.pop(0)
                    fn()
                    if tag:
                        done_tags.add(tag)

            def ensure(tag):
                while fillq and tag not in done_tags:
                    t, fn = fillq.pop(0)
                    fn()
                    if t:
                        done_tags.add(t)

            def qk_jobs(ft, tb):
                """3 micro-jobs: qkv projection for one (feature tile, token
                block), accumulating cc chunks into one aux-psum tile."""
                st = {}
                nm = f"fqk{ft}_{tb}"
                def mk(c0, c1, last=False):
                    def f():
                        if "t" not in st:
                            st["t"] = aux.tile([128, 512], f32, tag="aux",
                                               name=nm)
                        for cc in (c0, c1):
                            nc.tensor.matmul(
                                st["t"],
                                wqk_sb[:, cc, ft, :],
                                xt[:, cc, tb * 512:(tb + 1) * 512],
                                start=(cc == 0),
                                stop=(cc == CC - 1),
                            )
                        if last:
                            nc.vector.tensor_scalar_add(
                                out=qkt[:, ft, tb * 512:(tb + 1) * 512],
                                in0=st["t"],
                                scalar1=bqk_sb[:, ft:ft + 1],
                            )
                    return f
                tag = f"qk{ft}_{tb}"
                return [(None, mk(0, 1)), (None, mk(2, 3)),
                        (tag, mk(4, 5, last=True))]

            def v_jobs(pt):
                """4 micro-jobs: v projection for one token tile."""
                st = {}
                nm = f"fv{pt}"
                def mm(cc):
                    if "t" not in st:
                        st["t"] = aux.tile([128, 512], f32, tag="aux",
                                           name=nm)
                    vp = st["t"][:, 0:VW]
                    if cc < CC:
                        nc.tensor.matmul(
                            vp,
                            xt[:, cc, pt * 128:(pt + 1) * 128],
                            wv_sb[:, cc, :],
                            start=(cc == 0),
                            stop=False,
                        )
                    else:
                        nc.tensor.matmul(
                            vp,
                            xt1[0:1, pt * 128:(pt + 1) * 128],
                            wv1[0:1, :],
                            start=False,
                            stop=True,
                        )
                def mk(ccs, last=False):
                    def f():
                        for cc in ccs:
                            mm(cc)
                        if last:
                            vpv = st["t"][:, 0:VW].rearrange(
                                "p (h c) -> p h c", c=HD + 1)
                            vdst = vsb.rearrange(
                                "p a (h g) -> p a h g", g=VG)[:, pt]
                            nc.vector.tensor_copy(out=vdst[:, :, 0:1],
                                                  in_=vpv[:, :, 0:1])
                            nc.vector.tensor_copy(out=vdst[:, :, 64:128],
                                                  in_=vpv[:, :, 1:65])
                    return f
                tag = f"v{pt}"
                return [(None, mk((0, 1))), (None, mk((2, 3))),
                        (None, mk((4, 5))), (tag, mk((6,), last=True))]

            def proj_jobs(of, tb):
                """2 micro-jobs: output projection tile (of, tb)."""
                st = {}
                nm = f"fpj{of}_{tb}"
                def mk(fcs, last=False):
                    def f():
                        if "t" not in st:
                            st["t"] = aux.tile([128, 512], f32, tag="aux",
                                               name=nm)
                        for fc in fcs:
                            nc.tensor.matmul(
                                st["t"],
                                wp_sb[:, fc, of * 128:(of + 1) * 128],
                                otsb[:, fc, tb * 512:(tb + 1) * 512],
                                start=(fc == 0),
                                stop=(fc == 2),
                            )
                        if last:
                            ysl = norm.tile([128, 512], f32, tag="ysl",
                                            name=nm + "y")
                            nc.vector.tensor_scalar_add(
                                out=ysl, in0=st["t"],
                                scalar1=bp_sb[:, of:of + 1])
                            nc.sync.dma_start(
                                out=yT[of * 128:(of + 1) * 128,
                                       tb * 512:(tb + 1) * 512],
                                in_=ysl)
                    return f
                tag = f"pj{of}_{tb}"
                return [(None, mk((0, 1))), (tag, mk((2,), last=True))]

            # ---------------- attention building blocks ----------------
            def score_mm(p, qq, g, sp, off):
                """scores MM for block g=(kt, hd) -> sp col off"""
                kt, hd = g // 2, g % 2
                pb = 64 * hd
                nc.tensor.matmul(
                    sp[:, off:off + 512],
                    qkt[pb:pb + 64, 3 + p, kt * 128:(kt + 1) * 128],
                    qkt[pb:pb + 64, p, qq * 512:(qq + 1) * 512],
                    start=True,
                    stop=True,
                )

            def ot_mm(pp, hd, kc, pslab, op):
                ph = 2 * pp + hd
                nc.tensor.matmul(
                    op,
                    vsb[:, kc, ph * VG:(ph + 1) * VG],
                    pslab[:, kc * 2 + hd, :],
                    start=(kc == 0),
                    stop=(kc == KT - 1),
                )

            def ot_norm(pp, hd, qq, op):
                """normalize finished o^T psum [128, 512]; den at part 0,
                o^T at partitions 64:128 (VG layout)"""
                ph = 2 * pp + hd
                osb = norm.tile([128, 512], f32, tag="osb")
                nc.vector.tensor_copy(out=osb, in_=op)
                rec = norm.tile([1, 512], f32, tag="rec")
                rsc = norm.tile([1, 512], f32, tag="rsc")
                nc.vector.reciprocal_approx_accurate(
                    out=rec, in_=osb[0:1, :], scratch=rsc)
                # partition-broadcast via DRAM bounce (SBUF source cannot
                # have a zero partition step)
                dsc = drs.tile([1, 512], f32, tag="dsc")
                nc.sync.dma_start(out=dsc, in_=rec)
                rb = norm.tile([128, 512], f32, tag="rb")
                nc.gpsimd.dma_start(out=rb[64:128, :],
                                    in_=dsc.partition_broadcast(64))
                nc.vector.tensor_mul(
                    out=otsb[64 * (ph % 2):64 * (ph % 2) + 64,
                             ph // 2, qq * 512:(qq + 1) * 512],
                    in0=osb[64:128, :],
                    in1=rb[64:128, :],
                )

            # ---------------- build the fill queue ----------------
            for tb in (1, 2, 3):
                fillq += qk_jobs(0, tb)
            for pt in range(PT):
                fillq += v_jobs(pt)
            for tb in range(TB):
                fillq += qk_jobs(4, tb)
            for tb in range(TB):
                fillq += qk_jobs(1, tb)
            for tb in range(TB):
                fillq += qk_jobs(5, tb)
            for tb in range(TB):
                fillq += qk_jobs(2, tb)

            # ---------------- upfront: minimal qk proj ----------------
            for tb in range(TB):
                for tag, fn in qk_jobs(3, tb):
                    fn()
                    done_tags.add(tag)
            for tag, fn in qk_jobs(0, 0):
                fn()
                done_tags.add(tag)

            # ---------------- main pipeline ----------------
            # o^T drain pace per unit (32 jobs over 11 units)
            PACE = [3, 3, 3, 3, 3, 3, 3, 3, 3, 3, 2]
            prev = None       # (p, qq, slab) of the chunk being drained
            for c in range(12):
                p, qq = c // 4, c % 4
                # features this chunk's scores read must be emitted first
                ensure(f"qk{3 + p}_3")
                ensure(f"qk{p}_{qq}")
                if c == 10:
                    for of in range(6):
                        fillq += proj_jobs(of, 0)
                if c == 11:
                    for of in range(6):
                        fillq += proj_jobs(of, 1)
                slab = slabs.tile([128, NBLK, 512], bf16, tag="slab")
                ot_ps = {}
                if prev is not None:
                    pp, pqq, pslab = prev
                    # heads sequential: one aux slot holds the accumulator
                    ot_jobs = [(hd, kc) for hd in range(2)
                               for kc in range(KT)]
                for u in range(N_UNITS):
                    width = min(UNIT, TOTAL - u * UNIT)
                    if c == 1:
                        ensure(f"v{min(PT - 1, 3 * u + 3)}")
                    sp = psum_s.tile([128, UNIT], f32, tag="sp")
                    for j in range(width // 512):
                        score_mm(p, qq, u * 3 + j, sp, j * 512)
                    nc.scalar.activation(
                        out=slab.rearrange("p a b -> p (a b)")[
                            :, u * UNIT:u * UNIT + width],
                        in_=sp[:, 0:width],
                        func=AF.Exp,
                        scale=SCALE,
                    )
                    if prev is not None:
                        for _ in range(min(PACE[u], len(ot_jobs))):
                            hd, kc = ot_jobs.pop(0)
                            if hd not in ot_ps:
                                ot_ps[hd] = aux.tile(
                                    [128, 512], f32, tag="aux",
                                    name=f"op{c}_{hd}")
                            ot_mm(pp, hd, kc, pslab, ot_ps[hd])
                            if kc == KT - 1:
                                ot_norm(pp, hd, pqq, ot_ps.pop(hd))
                    n_fill = 2 if c == 0 else 1
                    pop_fill(n_fill)
                if prev is not None and ot_jobs:
                    while ot_jobs:
                        hd, kc = ot_jobs.pop(0)
                        if hd not in ot_ps:
                            ot_ps[hd] = aux.tile(
                                [128, 512], f32, tag="aux",
                                name=f"op{c}_{hd}")
                        ot_mm(pp, hd, kc, pslab, ot_ps[hd])
                        if kc == KT - 1:
                            ot_norm(pp, hd, pqq, ot_ps.pop(hd))
                prev = (p, qq, slab)

            # ---------------- tail: drain last chunk + proj ----------------
            for of in range(6):
                fillq += proj_jobs(of, 2)
            pp, pqq, pslab = prev
            ot_jobs = [(hd, kc) for hd in range(2) for kc in range(KT)]
            ot_ps = {}
            while ot_jobs:
                for _ in range(min(4, len(ot_jobs))):
                    hd, kc = ot_jobs.pop(0)
                    if hd not in ot_ps:
                        ot_ps[hd] = aux.tile([128, 512], f32, tag="aux",
                                             name=f"opt{hd}")
                    ot_mm(pp, hd, kc, pslab, ot_ps[hd])
                    if kc == KT - 1:
                        ot_norm(pp, hd, pqq, ot_ps.pop(hd))
                pop_fill(1)
            for of in range(6):
                fillq += proj_jobs(of, 3)
            pop_fill(len(fillq))
            assert not fillq

    nc.finalize()
    return nc


def _get_program():
    global _PROG
    if _PROG is None:
        _PROG = _build_program()
    return _PROG


def _prep_core_inputs(x, w_qkv, b_qkv, w_proj, b_proj, core):
    b, half = core // 2, core % 2
    heads = np.arange(H_LOC) + H_LOC * half  # global head ids
    d = np.arange(HD)

    import ml_dtypes
    bft = ml_dtypes.bfloat16
    xT = np.empty((769, 2048), bft)
    xT[:768] = x[b].T.astype(bft)
    xT[768] = 1.0

    # qk feature selection honoring torch reshape quirk: row = t*768 + d*12 + h
    # feature tiles: q(0,1) q(2,3) q(4,5) k(0,1) k(2,3) k(4,5)
    qk_rows = np.empty(768, np.int64)
    for j in range(3):  # head-pair tiles
        for hp in range(2):
            hh = heads[2 * j + hp]
            base = j * 128 + hp * 64
            qk_rows[base:base + 64] = d * 12 + hh           # q rows
            qk_rows[384 + base:384 + base + 64] = 768 + d * 12 + hh  # k rows
    wqk = np.ascontiguousarray(w_qkv[qk_rows].T.astype(bft))  # [768 c, 768 feat]
    bqk = np.ascontiguousarray(b_qkv[qk_rows].reshape(6, 128).T)  # [128, 6]

    wv = np.zeros((769, VW), bft)
    for i in range(H_LOC):
        rows = 1536 + d * 12 + heads[i]
        wv[768, 65 * i] = 1.0
        wv[:768, 65 * i + 1:65 * i + 65] = w_qkv[rows].T.astype(bft)
        wv[768, 65 * i + 1:65 * i + 65] = b_qkv[rows]

    wp = np.empty((384, 768), bft)
    for i in range(H_LOC):
        cols = 64 * heads[i] + d
        wp[64 * i:64 * i + 64] = w_proj[:, cols].T
    bp = np.ascontiguousarray((b_proj * 0.5).reshape(6, 128).T)

    return {
        "xT": xT,
        "wqk": wqk,
        "wv": np.ascontiguousarray(wv),
        "wp": np.ascontiguousarray(wp),
        "bqk": bqk,
        "bp": np.ascontiguousarray(bp),
    }


def _run(inputs, trace=False, **kw):
    from concourse.bass_utils import run_bass_kernel_spmd

    nc = _get_program()
    x = np.asarray(inputs["x"], np.float32)
    w_qkv = np.asarray(inputs["w_qkv"], np.float32)
    b_qkv = np.asarray(inputs["b_qkv"], np.float32)
    w_proj = np.asarray(inputs["w_proj"], np.float32)
    b_proj = np.asarray(inputs["b_proj"], np.float32)

    in_maps = [
        _prep_core_inputs(x, w_qkv, b_qkv, w_proj, b_proj, c)
        for c in range(N_CORES)
    ]
    res = run_bass_kernel_spmd(nc, in_maps, list(range(N_CORES)),
                               trace=trace, **kw)

    out = np.empty((B, P, D), np.float32)
    for b in range(B):
        yt = res.results[2 * b]["yT"] + res.results[2 * b + 1]["yT"]
        out[b] = yt.T
    return out, res


def kernel(**inputs):
    out, _ = _run(inputs)
    return out


# revision 13
# speedup vs baseline: 1.1954x; 1.1954x over previous
"""Trainium2 Bass kernel for nn_Attention (B=4, P=2048, D=768, H=12, hd=64).

Sharding: 8 cores = 4 batches x 2 half-head-groups (6 heads each).

Pipeline (per core): the Scalar engine's exp stream (~0.53 cyc/elem +
~900 cyc/instruction -> ~186us at 1536-col units) and the PE matmul stream
(~190us) are nearly equal, so the kernel runs one attention pipeline that
keeps both saturated end to end:
  - chunk = (head pair p, q-quarter qq); 12 chunks; 11 exp units per chunk
  - unit u: 3 score MMs (512-col blocks; the pair's heads alternate array
    row halves 0-63/64-127 and overlap on the PE) -> [128, 1536] PSUM ->
    one Exp ACTIVATE into the bf16 slab
  - o^T for chunk c-1 drains during chunk c (~3 MMs/unit, heads
    sequential so one aux-psum slot suffices), denominator via a
    ones-column in the V stationary (softmax without max subtraction,
    |s| <= 9.2); VG=128 stationary keeps fast-weight-load enabled
  - qkv q/k features, v projection and output projection stream through a
    deadline-tagged fill queue consumed in leftover PE slots each unit
PSUM: scores 2x[128,1536] (6 banks) + one 2-slot [128,512] aux pool shared
by the o^T accumulators and the fill-job tiles.
Host sums the two half-head partials per batch and transposes back.
"""

import sys

import numpy as np

if "/opt/trn_rl_repo" not in sys.path:
    sys.path.insert(0, "/opt/trn_rl_repo")

B, P, D = 4, 2048, 768
H, HD = 12, 64
N_CORES = 8
H_LOC = 6
SCALE = HD ** -0.5

CC = 6         # contraction chunks of 128 over D=768
KT = 16        # k-position tiles of 128 over P=2048
PT = 16        # token tiles of 128
TB = 4         # token blocks of 512
VG = 128
VW = H_LOC * (HD + 1)  # 390

UNIT = 1536
NBLK = 2 * KT          # 32 512-blocks per chunk
TOTAL = NBLK * 512
N_UNITS = (TOTAL + UNIT - 1) // UNIT  # 11 (last = 1024)

_PROG = None


def _build_program():
    import concourse.mybir as mybir
    import concourse.tile as tile
    from concourse import bacc

    f32 = mybir.dt.float32
    bf16 = mybir.dt.bfloat16
    AF = mybir.ActivationFunctionType

    nc = bacc.Bacc("TRN2")

    xT = nc.declare_dram_parameter("xT", [769, 2048], bf16, isOutput=False)
    wqk = nc.declare_dram_parameter("wqk", [768, 768], bf16, isOutput=False)
    wv = nc.declare_dram_parameter("wv", [768, VW], bf16, isOutput=False)
    wp = nc.declare_dram_parameter("wp", [384, 768], bf16, isOutput=False)
    bqk = nc.declare_dram_parameter("bqk", [128, 6], f32, isOutput=False)
    bp = nc.declare_dram_parameter("bp", [128, 6], f32, isOutput=False)
    yT = nc.declare_dram_parameter("yT", [768, 2048], f32, isOutput=True)

    with tile.TileContext(nc) as tc:
        with (
            tc.tile_pool(name="persist", bufs=1) as persist,
            tc.tile_pool(name="slabs", bufs=2) as slabs,
            tc.tile_pool(name="norm", bufs=3) as norm,
            tc.tile_pool(name="drs", bufs=4, space="DRAM") as drs,
            tc.tile_pool(name="psum_s", bufs=2, space="PSUM") as psum_s,
            tc.tile_pool(name="aux", bufs=2, space="PSUM") as aux,
        ):
            # ---- persistent SBUF tensors ----
            qkt = persist.tile([128, 6, 2048], bf16, tag="qkt")
            vsb = persist.tile([128, KT, H_LOC * VG], bf16, tag="vsb")
            nc.vector.memset(
                vsb.rearrange("p a (h g) -> p a h g", g=VG)[:, :, :, 0:1], 1.0)
            nc.vector.memset(
                vsb.rearrange("p a (h g) -> p a h g", g=VG)[:, :, :, 1:64], 0.0)
            otsb = persist.tile([128, 3, 2048], bf16, tag="otsb")
            xt = persist.tile([128, CC, 2048], bf16, tag="xt")
            wqk_sb = persist.tile([128, CC, 6, 128], bf16, tag="wqk_sb")
            wv_sb = persist.tile([128, CC, VW], bf16, tag="wv_sb")
            wp_sb = persist.tile([128, 3, 768], bf16, tag="wp_sb")
            bqk_sb = persist.tile([128, 6], f32, tag="bqk_sb")
            bp_sb = persist.tile([128, 6], f32, tag="bp_sb")

            nc.sync.dma_start(out=bqk_sb, in_=bqk[:, :])
            nc.sync.dma_start(out=bp_sb, in_=bp[:, :])
            # pre-warm the exp ACT table set (~2.7us) during the DMA lead
            warmup = persist.tile([1, 1], f32, tag="warmup")
            nc.vector.memset(warmup, 0.0)
            nc.scalar.activation(out=warmup, in_=warmup, func=AF.Exp)

            # DMA chase order: per cc bring x, then the k/q features of
            # pair 0 (ft3, ft0) so the first chunk can start ASAP.
            for cc in range(CC):
                nc.sync.dma_start(out=xt[:, cc, :],
                                  in_=xT[cc * 128:(cc + 1) * 128, :])
                for ft in (3, 0):
                    nc.sync.dma_start(
                        out=wqk_sb[:, cc, ft, :],
                        in_=wqk[cc * 128:(cc + 1) * 128,
                                ft * 128:(ft + 1) * 128])
            for cc in range(CC):
                nc.sync.dma_start(out=wv_sb[:, cc, :],
                                  in_=wv[cc * 128:(cc + 1) * 128, :])
            for ft in (4, 1, 5, 2):
                for cc in range(CC):
                    nc.sync.dma_start(
                        out=wqk_sb[:, cc, ft, :],
                        in_=wqk[cc * 128:(cc + 1) * 128,
                                ft * 128:(ft + 1) * 128])
            for fc in range(3):
                nc.sync.dma_start(out=wp_sb[:, fc, :],
                                  in_=wp[fc * 128:(fc + 1) * 128, :])

            # ---------------- fill-job machinery ----------------
            # Small closures (<= 2 matmuls each) consumed in per-unit slack.
            fillq = []       # list of (tag, fn)
            done_tags = set()

            def pop_fill(n):
                for _ in range(n):
                    if not fillq:
                        return
                    tag, fn = fillq.pop(0)
                    fn()
                    if tag:
                        done_tags.add(tag)

            def ensure(tag):
                while fillq and tag not in done_tags:
                    t, fn = fillq.pop(0)
                    fn()
                    if t:
                        done_tags.add(t)

            def qk_jobs(ft, tb):
                """3 micro-jobs: qkv projection for one (feature tile, token
                block), accumulating cc chunks into one aux-psum tile."""
                st = {}
                nm = f"fqk{ft}_{tb}"
                def mk(c0, c1, last=False):
                    def f():
                        if "t" not in st:
                            st["t"] = aux.tile([128, 512], f32, tag="aux",
                                               name=nm)
                        for cc in (c0, c1):
                            nc.tensor.matmul(
                                st["t"],
                                wqk_sb[:, cc, ft, :],
                                xt[:, cc, tb * 512:(tb + 1) * 512],
                                start=(cc == 0),
                                stop=(cc == CC - 1),
                            )
                        if last:
                            nc.vector.tensor_scalar_add(
                                out=qkt[:, ft, tb * 512:(tb + 1) * 512],
                                in0=st["t"],
                                scalar1=bqk_sb[:, ft:ft + 1],
                            )
                    return f
                tag = f"qk{ft}_{tb}"
                return [(None, mk(0, 1)), (None, mk(2, 3)),
                        (tag, mk(4, 5, last=True))]

            def v_jobs(pt):
                """3 micro-jobs: v projection for one token tile. The v bias
                is folded into bp on host (softmax rows sum to 1, so
                Wp@(o+b_v) = Wp@o + Wp@b_v); the denominator ones column is
                memset once at startup."""
                st = {}
                nm = f"fv{pt}"
                def mk(ccs, last=False):
                    def f():
                        if "t" not in st:
                            st["t"] = aux.tile([128, 512], f32, tag="aux",
                                               name=nm)
                        for cc in ccs:
                            nc.tensor.matmul(
                                st["t"][:, 0:VW],
                                xt[:, cc, pt * 128:(pt + 1) * 128],
                                wv_sb[:, cc, :],
                                start=(cc == 0),
                                stop=(cc == CC - 1),
                            )
                        if last:
                            vpv = st["t"][:, 0:VW].rearrange(
                                "p (h c) -> p h c", c=HD + 1)
                            vdst = vsb.rearrange(
                                "p a (h g) -> p a h g", g=VG)[:, pt]
                            nc.vector.tensor_copy(out=vdst[:, :, 64:128],
                                                  in_=vpv[:, :, 1:65])
                    return f
                tag = f"v{pt}"
                return [(None, mk((0, 1))), (None, mk((2, 3))),
                        (tag, mk((4, 5), last=True))]

            def proj_jobs(of, tb, pool=None, ptag="aux"):
                """2 micro-jobs: output projection tile (of, tb)."""
                st = {}
                nm = f"fpj{of}_{tb}"
                pl = pool if pool is not None else aux
                def mk(fcs, last=False):
                    def f():
                        if "t" not in st:
                            st["t"] = pl.tile([128, 512], f32, tag=ptag,
                                              name=nm)
                        for fc in fcs:
                            nc.tensor.matmul(
                                st["t"],
                                wp_sb[:, fc, of * 128:(of + 1) * 128],
                                otsb[:, fc, tb * 512:(tb + 1) * 512],
                                start=(fc == 0),
                                stop=(fc == 2),
                            )
                        if last:
                            ysl = norm.tile([128, 512], f32, tag="ysl",
                                            name=nm + "y")
                            nc.vector.tensor_scalar_add(
                                out=ysl, in0=st["t"],
                                scalar1=bp_sb[:, of:of + 1])
                            nc.sync.dma_start(
                                out=yT[of * 128:(of + 1) * 128,
                                       tb * 512:(tb + 1) * 512],
                                in_=ysl)
                    return f
                tag = f"pj{of}_{tb}"
                return [(None, mk((0, 1))), (tag, mk((2,), last=True))]

            # ---------------- attention building blocks ----------------
            def score_mm(p, qq, g, sp, off):
                """scores MM for block g=(kt, hd) -> sp col off"""
                kt, hd = g // 2, g % 2
                pb = 64 * hd
                nc.tensor.matmul(
                    sp[:, off:off + 512],
                    qkt[pb:pb + 64, 3 + p, kt * 128:(kt + 1) * 128],
                    qkt[pb:pb + 64, p, qq * 512:(qq + 1) * 512],
                    start=True,
                    stop=True,
                )

            def ot_mm(pp, hd, kc, pslab, op):
                ph = 2 * pp + hd
                nc.tensor.matmul(
                    op,
                    vsb[:, kc, ph * VG:(ph + 1) * VG],
                    pslab[:, kc * 2 + hd, :],
                    start=(kc == 0),
                    stop=(kc == KT - 1),
                )

            def ot_norm(pp, hd, qq, op):
                """normalize finished o^T psum [128, 512]; den at part 0,
                o^T at partitions 64:128 (VG layout)"""
                ph = 2 * pp + hd
                osb = norm.tile([128, 512], f32, tag="osb")
                nc.vector.tensor_copy(out=osb, in_=op)
                rec = norm.tile([1, 512], f32, tag="rec")
                rsc = norm.tile([1, 512], f32, tag="rsc")
                nc.vector.reciprocal_approx_accurate(
                    out=rec, in_=osb[0:1, :], scratch=rsc)
                # partition-broadcast via DRAM bounce (SBUF source cannot
                # have a zero partition step)
                dsc = drs.tile([1, 512], f32, tag="dsc")
                nc.sync.dma_start(out=dsc, in_=rec)
                rb = norm.tile([128, 512], f32, tag="rb")
                nc.gpsimd.dma_start(out=rb[64:128, :],
                                    in_=dsc.partition_broadcast(64))
                nc.vector.tensor_mul(
                    out=otsb[64 * (ph % 2):64 * (ph % 2) + 64,
                             ph // 2, qq * 512:(qq + 1) * 512],
                    in0=osb[64:128, :],
                    in1=rb[64:128, :],
                )

            # ---------------- build the fill queue ----------------
            for tb in (1, 2, 3):
                fillq += qk_jobs(0, tb)
            for pt in range(PT):
                fillq += v_jobs(pt)
            for tb in range(TB):
                fillq += qk_jobs(4, tb)
            for tb in range(TB):
                fillq += qk_jobs(1, tb)
            for tb in range(TB):
                fillq += qk_jobs(5, tb)
            for tb in range(TB):
                fillq += qk_jobs(2, tb)

            # ---------------- upfront: minimal qk proj ----------------
            for tb in range(TB):
                for tag, fn in qk_jobs(3, tb):
                    fn()
                    done_tags.add(tag)
            for tag, fn in qk_jobs(0, 0):
                fn()
                done_tags.add(tag)

            # ---------------- main pipeline ----------------
            # o^T drain pace per unit (32 jobs over 11 units)
            PACE = [3, 3, 3, 3, 3, 3, 3, 3, 3, 3, 2]
            PACE_LAST = [4, 4, 4, 4, 4, 4, 4, 4, 0, 0, 0]
            prev = None       # (p, qq, slab) of the chunk being drained
            for c in range(12):
                p, qq = c // 4, c % 4
                # features this chunk's scores read must be emitted first
                ensure(f"qk{3 + p}_3")
                ensure(f"qk{p}_{qq}")
                if c == 10:
                    for of in range(6):
                        fillq += proj_jobs(of, 0)
                if c == 11:
                    for of in range(6):
                        fillq += proj_jobs(of, 1)
                slab = slabs.tile([128, NBLK, 512], bf16, tag="slab")
                ot_ps = {}
                if prev is not None:
                    pp, pqq, pslab = prev
                    # heads sequential: one aux slot holds the accumulator
                    ot_jobs = [(hd, kc) for hd in range(2)
                               for kc in range(KT)]
                pace = PACE_LAST if c == 11 else PACE
                for u in range(N_UNITS):
                    width = min(UNIT, TOTAL - u * UNIT)
                    if c == 1:
                        ensure(f"v{min(PT - 1, 3 * u + 3)}")
                    sp = psum_s.tile([128, UNIT], f32, tag="sp")
                    for j in range(width // 512):
                        score_mm(p, qq, u * 3 + j, sp, j * 512)
                    nc.scalar.activation(
                        out=slab.rearrange("p a b -> p (a b)")[
                            :, u * UNIT:u * UNIT + width],
                        in_=sp[:, 0:width],
                        func=AF.Exp,
                        scale=SCALE,
                    )
                    if prev is not None:
                        for _ in range(min(pace[u], len(ot_jobs))):
                            hd, kc = ot_jobs.pop(0)
                            if hd not in ot_ps:
                                ot_ps[hd] = aux.tile(
                                    [128, 512], f32, tag="aux",
                                    name=f"op{c}_{hd}")
                            ot_mm(pp, hd, kc, pslab, ot_ps[hd])
                            if kc == KT - 1:
                                ot_norm(pp, hd, pqq, ot_ps.pop(hd))
                    n_fill = {0: 3, 1: 1, 2: 2, 3: 2}.get(c, 1)
                    if c >= 10:
                        n_fill = 2
                    pop_fill(n_fill)
                if prev is not None and ot_jobs:
                    while ot_jobs:
                        hd, kc = ot_jobs.pop(0)
                        if hd not in ot_ps:
                            ot_ps[hd] = aux.tile(
                                [128, 512], f32, tag="aux",
                                name=f"op{c}_{hd}")
                        ot_mm(pp, hd, kc, pslab, ot_ps[hd])
                        if kc == KT - 1:
                            ot_norm(pp, hd, pqq, ot_ps.pop(hd))
                prev = (p, qq, slab)

            # ---------------- tail: drain last chunk + proj ----------------
            # tail proj tiles borrow the score-psum slots (free after the
            # final exp) so up to 4 proj groups pipeline with the drain
            for of in range(6):
                fillq += proj_jobs(of, 2, pool=psum_s, ptag="sp")
            pp, pqq, pslab = prev
            ot_jobs = [(hd, kc) for hd in range(2) for kc in range(KT)]
            ot_ps = {}
            while ot_jobs:
                for _ in range(min(4, len(ot_jobs))):
                    hd, kc = ot_jobs.pop(0)
                    if hd not in ot_ps:
                        ot_ps[hd] = aux.tile([128, 512], f32, tag="aux",
                                             name=f"opt{hd}")
                    ot_mm(pp, hd, kc, pslab, ot_ps[hd])
                    if kc == KT - 1:
                        ot_norm(pp, hd, pqq, ot_ps.pop(hd))
                pop_fill(2)
            for of in range(6):
                fillq += proj_jobs(of, 3)
            pop_fill(len(fillq))
            assert not fillq

    nc.finalize()
    return nc


def _get_program():
    global _PROG
    if _PROG is None:
        _PROG = _build_program()
    return _PROG


def _prep_core_inputs(x, w_qkv, b_qkv, w_proj, b_proj, core):
    b, half = core // 2, core % 2
    heads = np.arange(H_LOC) + H_LOC * half  # global head ids
    d = np.arange(HD)

    import ml_dtypes
    bft = ml_dtypes.bfloat16
    xT = np.empty((769, 2048), bft)
    xT[:768] = x[b].T.astype(bft)
    xT[768] = 1.0

    # qk feature selection honoring torch reshape quirk: row = t*768 + d*12 + h
    # feature tiles: q(0,1) q(2,3) q(4,5) k(0,1) k(2,3) k(4,5)
    qk_rows = np.empty(768, np.int64)
    for j in range(3):  # head-pair tiles
        for hp in range(2):
            hh = heads[2 * j + hp]
            base = j * 128 + hp * 64
            qk_rows[base:base + 64] = d * 12 + hh           # q rows
            qk_rows[384 + base:384 + base + 64] = 768 + d * 12 + hh  # k rows
    wqk = np.ascontiguousarray(w_qkv[qk_rows].T.astype(bft))  # [768 c, 768 feat]
    bqk = np.ascontiguousarray(b_qkv[qk_rows].reshape(6, 128).T)  # [128, 6]

    wv = np.zeros((768, VW), bft)
    bv_fold = np.zeros(768, np.float64)
    for i in range(H_LOC):
        rows = 1536 + d * 12 + heads[i]
        wv[:, 65 * i + 1:65 * i + 65] = w_qkv[rows].T.astype(bft)
        cols = 64 * heads[i] + d
        bv_fold += w_proj[:, cols].astype(np.float64) @ b_qkv[rows]

    wp = np.empty((384, 768), bft)
    for i in range(H_LOC):
        cols = 64 * heads[i] + d
        wp[64 * i:64 * i + 64] = w_proj[:, cols].T
    bp = np.ascontiguousarray(
        (b_proj * 0.5 + bv_fold).astype(np.float32).reshape(6, 128).T)

    return {
        "xT": xT,
        "wqk": wqk,
        "wv": np.ascontiguousarray(wv),
        "wp": np.ascontiguousarray(wp),
        "bqk": bqk,
        "bp": np.ascontiguousarray(bp),
    }


def _run(inputs, trace=False, **kw):
    from concourse.bass_utils import run_bass_kernel_spmd

    nc = _get_program()
    x = np.asarray(inputs["x"], np.float32)
    w_qkv = np.asarray(inputs["w_qkv"], np.float32)
    b_qkv = np.asarray(inputs["b_qkv"], np.float32)
    w_proj = np.asarray(inputs["w_proj"], np.float32)
    b_proj = np.asarray(inputs["b_proj"], np.float32)

    in_maps = [
        _prep_core_inputs(x, w_qkv, b_qkv, w_proj, b_proj, c)
        for c in range(N_CORES)
    ]
    res = run_bass_kernel_spmd(nc, in_maps, list(range(N_CORES)),
                               trace=trace, **kw)

    out = np.empty((B, P, D), np.float32)
    for b in range(B):
        yt = res.results[2 * b]["yT"] + res.results[2 * b + 1]["yT"]
        out[b] = yt.T
    return out, res


def kernel(**inputs):
    out, _ = _run(inputs)
    return out
